# revision 9
# baseline (speedup 1.0000x reference)
"""Trainium2 Bass kernel for nn_Conv2Central (S^4 separable stencil).

The reference computes y = S(rev(S(S(rev(S(x)))))) where S is the 2x2
stencil  out[i,j] = x[i,j] + .5 x[i,j+1] + .5 x[i+1,j] + .25 x[i+1,j+1]
(zero-padded bottom/right) applied per image, and rev reverses the batch.
S acts independently per image, so it commutes with any batch permutation:
the two reversals cancel exactly and the whole network is S^4 — a
separable 5-tap forward filter K = [1, 2, 1.5, 0.5, 0.0625] applied along
H then W with zero padding at the bottom/right ([1,.5] convolved with
itself 4 times).

Sharding: batch N=32 split across the 8 NeuronCores (4 images per core),
no inter-core communication needed.

Per-core implementation (fp16 on-device; the 2e-2 harness gate leaves
~20x margin over fp16's ~1e-3 end-to-end error, and fp16 halves both HBM
traffic and doubles DVE tensor_scalar/copy throughput vs fp32): the 4
images are stacked into a [4096, 1024] strip and tiled in 128-row input
tiles with stride 124 (4-row overlap feeds the vertical taps).

Work split per tile (cost-model ns in brackets; DMA_ENGINES is a shared
~360 GB/s pool, so in+out ~1456 ns/tile is the roofline):
  DMA  input rides the SP HWDGE queue [728 on DMA_ENGINES]; output is
       issued by gpsimd through SWDGE [728] so the per-DMA HWDGE fixed
       cost (~630 ns) is split across two generators.
  PE   fp16 banded matmuls accumulate in fp32 PSUM: A.T @ h plus one
       512-wide matmul pair per remaining horizontal tap d, using the
       scaled band (K5[d]*A).T @ x(+d), where A[k,m] = K5[k-m] is the
       vertical 5-diagonal band [213/matmul].
  DVE  builds h, the partial horizontal sum. A fused
       scalar_tensor_tensor MAC runs 1 elem/lane/cycle [1127] while
       tensor_scalar runs 4x [372] and tensor_tensor 2x [638], so
       tiles alternate between two variants, greedily balancing
       cumulative DVE vs PE time at ~1.5 us/tile each:
         A: h = STT(2*x(+1)+x); h = STT(1.5*x(+2)+h); PE taps {3,4}
         B: h = TT(x + TS(2*x(+1)));             PE taps {2,3,4}
  ACT  copy PSUM (fp32) -> SBUF (fp16) [1038], POOL pad memset.
Tiles straddling an image boundary use band matrices with cross-image
entries zeroed; the final tile outputs 128 rows (taps truncate at the
image bottom edge). All band weights are exact in fp16, so the only
error is fp16 quantization of the data path.
"""
import numpy as np

import concourse.bass as bass
import concourse.mybir as mybir
from concourse.tile import TileContext
from concourse.bass_utils import run_bass_kernel_spmd
from bass_rust import ScopedClock

N_CORES = 8
B = 4            # images per core
H = 1024
W = 1024
STRIDE = 124
K5 = [1.0, 2.0, 1.5, 0.5, 0.0625]
TAP4_PE_FRAC = 1.0  # kept for harness compat; tap 4 always runs on PE now
NP_DT = np.float16

# ---------------------------------------------------------------------------
# Workarounds for this container's walrus build, which rejects any
# instruction carrying more than ONE sync wait ("Too many sync wait
# commands").  (1) TileContext's tail drain aggregates a wait per live
# semaphore — replace it with a chain of sync NOPs, one wait each.
# (2) A general pass splits any remaining multi-wait instruction by
# hoisting extra waits onto same-engine NoOps inserted right before it
# (engine queues are FIFO, so the waits still complete first).
# ---------------------------------------------------------------------------


def _patched_drain_and_barrier(self, tick_clock, wait_clock):
    nc = self.nc
    probe = nc.sync.nop()
    wait_clock.add_sem_waits(probe.ins, ScopedClock({None: tick_clock.global_clock}))
    si = probe.ins.sync_info
    waits = list(si.on_wait) if si and si.on_wait else []
    if si is not None:
        si.on_wait = waits[:1]
    for i in range(1, len(waits)):
        n = nc.sync.nop()
        nsi = n.ins.sync_info
        if nsi is None:
            n.ins.sync_info = mybir.SyncInfo(on_wait=[waits[i]], on_update=[])
        else:
            nsi.on_wait = [waits[i]]
    nc.sync.drain()
    nc.all_engine_barrier()
    assert self.sems is not None
    popped = nc._tile_sem_poison_stack.pop()
    assert popped is self._sem_poison
    nc.clear_and_free_semaphores(list(self.sems.allocated().values()))
    nc.all_engine_barrier()


TileContext._drain_and_barrier = _patched_drain_and_barrier

_nop_counter = [0]


def _legalize_waits(nc):
    for f in nc.m.functions:
        for blk in f.blocks:
            out = []
            for inst in blk.instructions:
                si = inst.sync_info
                waits = list(si.on_wait) if si is not None and si.on_wait else []
                if len(waits) > 1:
                    for w in waits[:-1]:
                        _nop_counter[0] += 1
                        nop = mybir.InstNoOp(name=f"legalize-wait-{_nop_counter[0]}")
                        nop.engine = inst.engine
                        nop.sync_info = mybir.SyncInfo(on_wait=[w], on_update=[])
                        out.append(nop)
                    si.on_wait = [waits[-1]]
                out.append(inst)
            blk.instructions = out
    return nc


# ---------------------------------------------------------------------------
# Weights: banded vertical-filter matrices.
# ---------------------------------------------------------------------------


def _band_np(rows_in, rows_out, boundary=None):
    """A[k, m] = K5[k-m], zeroed where out-row m and in-row k straddle
    `boundary` (tile-local image split)."""
    A = np.zeros((rows_in, rows_out), dtype=np.float32)
    for m in range(rows_out):
        for d in range(5):
            k = m + d
            if k < rows_in and not (boundary is not None and m < boundary <= k):
                A[k, m] = K5[d]
    return A


def _tile_plan():
    """[(r0, pin, pout, boundary_or_None)] covering B*H rows."""
    total = B * H
    plan = []
    r0 = 0
    while r0 < total:
        if total - r0 <= 128:
            plan.append((r0, total - r0, total - r0, None))
            break
        boundary = None
        for k in range(1, B):
            if r0 < k * H < r0 + 128:
                boundary = k * H - r0
        plan.append((r0, 128, STRIDE, boundary))
        r0 += STRIDE
    return plan


SCALES = {"main": 1.0, "t2": 1.5, "t3": 0.5, "t4": 0.0625}


def _weights_np():
    plan = _tile_plan()
    classes = sorted({b for (_, _, _, b) in plan if b is not None})
    cols = []
    offs = {}

    def add(name, arr):
        offs[name] = sum(c.shape[1] for c in cols)
        cols.append(arr)

    for nm, s in SCALES.items():
        add(nm, s * _band_np(128, 128))
        for b in classes:
            add(f"{nm}{b}", s * _band_np(128, STRIDE, boundary=b))
    return np.concatenate(cols, axis=1).astype(NP_DT), offs


# Cost-model ns for the greedy DVE/PE balance (see module docstring).
_C_STT, _C_TS, _C_TT, _C_MM = 1127, 372, 638, 213


def _tile_types(n):
    """Per-tile variant choice: 'A' (2 STT, 6 matmuls) or 'B'
    (TS+TT, 8 matmuls), greedily balancing cumulative DVE vs PE."""
    types = []
    dve = pe = 0.0
    for _ in range(n):
        a_cost = max(dve + 2 * _C_STT, pe + 6 * _C_MM)
        b_cost = max(dve + _C_TS + _C_TT, pe + 8 * _C_MM)
        if a_cost <= b_cost:
            types.append("A")
            dve += 2 * _C_STT
            pe += 6 * _C_MM
        else:
            types.append("B")
            dve += _C_TS + _C_TT
            pe += 8 * _C_MM
    return types


# ---------------------------------------------------------------------------
# Kernel builder.
# ---------------------------------------------------------------------------


def _build(reps=1):
    nc = bass.Bass(trn_type="TRN2")
    DT = mybir.dt.float16
    PS = mybir.dt.float32
    STT = mybir.AluOpType
    pack, offs = _weights_np()
    x = nc.dram_tensor("x", [B, H, W], DT, kind="ExternalInput")
    wp = nc.dram_tensor("wpack", list(pack.shape), DT, kind="ExternalInput")
    y = nc.dram_tensor("y", [B, H, W], DT, kind="ExternalOutput")
    xf = x.rearrange("b h w -> (b h) w")
    yf = y.rearrange("b h w -> (b h) w")
    if reps > 1:
        scratch = nc.dram_tensor("scratch", [B, H, W], DT, kind="ExternalOutput")
        sf = scratch.rearrange("b h w -> (b h) w")

    plan = _tile_plan()
    types = _tile_types(len(plan))

    with TileContext(nc) as tc:
        with tc.tile_pool(name="wpool", bufs=1) as wpool, \
             tc.tile_pool(name="xp", bufs=8) as xp, \
             tc.tile_pool(name="hp", bufs=8) as hp, \
             tc.tile_pool(name="op", bufs=6) as op, \
             tc.tile_pool(name="pp", bufs=4, space="PSUM") as pp:
            wt = wpool.tile(list(pack.shape), DT)
            nc.sync.dma_start(out=wt[:], in_=wp[:])

            def wslice(name, bnd, pin, pout):
                o = offs[name if bnd is None else f"{name}{bnd}"]
                return wt[:pin, o:o + pout]

            for rep in range(reps):
              out_f = yf if rep == 0 else sf
              for ti, (r0, pin, pout, bnd) in enumerate(plan):
                xt = xp.tile([128, W + 4], DT, tag="xt")
                nc.sync.dma_start(out=xt[:pin, 0:W], in_=xf[r0:r0 + pin, :])
                nc.gpsimd.memset(xt[:pin, W:W + 4], 0)
                if types[ti] == "A":
                    hA = hp.tile([128, W], DT, tag="hA")
                    hB = hp.tile([128, W], DT, tag="hB")
                    nc.vector.scalar_tensor_tensor(
                        hA[:pin], xt[:pin, 1:W + 1], 2.0,
                        xt[:pin, 0:W], STT.mult, STT.add)
                    nc.vector.scalar_tensor_tensor(
                        hB[:pin], xt[:pin, 2:W + 2], 1.5,
                        hA[:pin], STT.mult, STT.add)
                    hin, pe_taps = hB, ("t3", "t4")
                else:
                    hS = hp.tile([128, W], DT, tag="hS")
                    hA = hp.tile([128, W], DT, tag="hA")
                    nc.vector.tensor_scalar_mul(
                        hS[:pin], xt[:pin, 1:W + 1], 2.0)
                    nc.vector.tensor_add(
                        hA[:pin], xt[:pin, 0:W], hS[:pin])
                    hin, pe_taps = hA, ("t2", "t3", "t4")
                ps = pp.tile([128, W], PS, tag="ps")
                last = len(pe_taps)
                for h in range(2):
                    nc.tensor.matmul(ps[:pout, h * 512:(h + 1) * 512],
                                     wslice("main", bnd, pin, pout),
                                     hin[:pin, h * 512:h * 512 + 512],
                                     start=True, stop=False)
                for wi, nm in enumerate(pe_taps):
                    d = int(nm[1])
                    for h in range(2):
                        nc.tensor.matmul(
                            ps[:pout, h * 512:(h + 1) * 512],
                            wslice(nm, bnd, pin, pout),
                            xt[:pin, d + h * 512:d + h * 512 + 512],
                            start=False, stop=(wi == last - 1))
                ot = op.tile([128, W], DT, tag="ot")
                nc.scalar.copy(ot[:pout], ps[:pout])
                nc.gpsimd.dma_start(out=out_f[r0:r0 + pout, :], in_=ot[:pout])
    _legalize_waits(nc)
    return nc


_CACHE = {}


def kernel(img: np.ndarray) -> np.ndarray:
    assert img.shape == (N_CORES * B, H, W), img.shape
    img16 = np.ascontiguousarray(np.asarray(img)).astype(NP_DT)
    if "nc" not in _CACHE:
        _CACHE["nc"] = _build()
        _CACHE["wpack"], _ = _weights_np()
    nc = _CACHE["nc"]
    pack = _CACHE["wpack"]
    in_maps = [{"x": img16[c * B:(c + 1) * B], "wpack": pack}
               for c in range(N_CORES)]
    res = run_bass_kernel_spmd(nc, in_maps, core_ids=list(range(N_CORES)))
    out = np.concatenate([res.results[c]["y"] for c in range(N_CORES)], axis=0)
    return out.astype(np.float32)
